# revision 1
# baseline (speedup 1.0000x reference)
"""Distributed FWHT (Hamiltonian -> Pauli-string coefficients) on 8 TRN2 cores.

Computes y = FWHT(x) / N for N = 2^24, sharded contiguously across 8 cores
(2^21 elements each).  FWHT = H8 (core axis) (x) H128 (x) H128 (x) H128.

Per-core kernel:
  - 3 local PE passes, each a "data-stationary" matmul sweep: 128 chunks of
    [128,128]; lhsT = data chunk, rhs = H128/128 -> out = chunk.T @ Hs.
    Each pass transforms the partition axis and rotates the layout so the
    next 7-bit axis lands on partitions.  All matmul reads are contiguous;
    the PSUM->SBUF copies write strided (runs of 4B @ 512B stride).
  - AllToAll across the 8 cores (chunked on the leading 7-bit axis).
  - One final PE pass with stationary kron(H8, I16)/8 over partitions
    (c', a'sub), combining the 8 cores' contributions.
Scaling by 1/2^24 is folded into the transform matrices (exact powers of 2).
"""

import math

import numpy as np

NCORES = 8
P = 128
F = 16384  # free elements per partition (2^21 per core / 128)
LOCAL = P * F


def _hadamard(n: int) -> np.ndarray:
    H = np.array([[1.0]], dtype=np.float64)
    while H.shape[0] < n:
        H = np.block([[H, H], [H, -H]])
    return H


_BUILD_CACHE: dict = {}


def _build_module():
    """Build + schedule the Bass module once per process."""
    if "nc" in _BUILD_CACHE:
        return _BUILD_CACHE["nc"]

    import concourse.bass as bass
    import concourse.mybir as mybir
    import concourse.tile as tile
    from concourse import bacc

    f32 = mybir.dt.float32

    Hs_np = (_hadamard(128) / 128.0).astype(np.float32)
    M_np = (np.kron(_hadamard(8), np.eye(16)) / 8.0).astype(np.float32)

    nc = bacc.Bacc(
        "TRN2",
        target_bir_lowering=False,
        debug=False,
        enable_asserts=False,
        num_devices=NCORES,
    )

    x_in = nc.dram_tensor("x", [P, F], f32, kind="ExternalInput")
    y_out = nc.dram_tensor("y", [P, F], f32, kind="ExternalOutput")
    Hs_dram = nc.inline_tensor(Hs_np, name="Hs_const")
    M_dram = nc.inline_tensor(M_np, name="M_const")

    with tile.TileContext(nc) as tc:
        with (
            tc.tile_pool(name="big", bufs=2) as big,
            tc.tile_pool(name="consts", bufs=1) as consts,
            tc.tile_pool(name="psum", bufs=6, space="PSUM") as psum,
            tc.tile_pool(name="dram", bufs=1, space="DRAM") as dram,
        ):
            Hs_t = consts.tile([P, 128], f32, tag="hs")
            M_t = consts.tile([P, 128], f32, tag="m")
            nc.sync.dma_start(Hs_t[:], Hs_dram[:])
            nc.sync.dma_start(M_t[:], M_dram[:])

            a2a_in = dram.tile([P, F], f32, tag="a2a_in")
            a2a_out = dram.tile([P, F], f32, tag="a2a_out")

            X = big.tile([P, F], f32, tag="big")
            # load input in 4 column blocks so pass 1 can start early
            for k in range(4):
                nc.sync.dma_start(
                    X[:, k * 4096 : (k + 1) * 4096], x_in[:, k * 4096 : (k + 1) * 4096]
                )

            def pass_sweep(src, dst, stationary):
                """One data-stationary FWHT pass: 128 chunk matmuls + copies.

                src layout [p; (u, v)]; chunk i = src[:, 128i:128i+128];
                out[v'; p'] written to dst cols {p'*128 + i} (dst [v'?]...
                layout [chunk-free; (p', i)]).
                """
                dst_r = dst[:].rearrange("p (a b) -> p b a", b=128)
                for g in range(32):
                    pt = psum.tile([P, 512], f32, tag="ps")
                    for j in range(4):
                        i = g * 4 + j
                        nc.tensor.matmul(
                            pt[:, j * 128 : (j + 1) * 128],
                            src[:, i * 128 : (i + 1) * 128],
                            stationary[:],
                        )
                    eng = nc.vector.tensor_copy if g % 2 == 0 else nc.scalar.copy
                    eng(
                        dst_r[:, g * 4 : (g + 1) * 4, :],
                        pt[:].rearrange("p (j a) -> p j a", j=4),
                    )

            Y = big.tile([P, F], f32, tag="big")
            pass_sweep(X, Y, Hs_t)
            Z = big.tile([P, F], f32, tag="big")
            pass_sweep(Y, Z, Hs_t)
            W = big.tile([P, F], f32, tag="big")
            pass_sweep(Z, W, Hs_t)

            nc.sync.dma_start(a2a_in[:], W[:])
            nc.gpsimd.collective_compute(
                "AllToAll",
                mybir.AluOpType.bypass,
                replica_groups=[list(range(NCORES))],
                ins=[a2a_in.opt()],
                outs=[a2a_out.opt()],
            )

            V = big.tile([P, F], f32, tag="big")
            nc.sync.dma_start(V[:], a2a_out[:])

            O = big.tile([P, F], f32, tag="big")
            for g in range(32):
                pt = psum.tile([P, 512], f32, tag="ps")
                nc.tensor.matmul(pt[:], M_t[:], V[:, g * 512 : (g + 1) * 512])
                eng = nc.vector.tensor_copy if g % 2 == 0 else nc.scalar.copy
                eng(O[:, g * 512 : (g + 1) * 512], pt[:])

            nc.sync.dma_start(y_out[:], O[:])

    nc.compile()
    _BUILD_CACHE["nc"] = nc
    return nc


def run(x: np.ndarray, trace: bool = False):
    """Run the 8-core kernel on the full input vector.

    Returns (y_full, BassKernelResults)."""
    from concourse.bass_utils import run_bass_kernel_spmd

    nc = _build_module()
    x = np.ascontiguousarray(x, dtype=np.float32)
    assert x.shape == (NCORES * LOCAL,)
    shards = x.reshape(NCORES, P, F)
    in_maps = [{"x": shards[c]} for c in range(NCORES)]
    res = run_bass_kernel_spmd(
        nc, in_maps, core_ids=list(range(NCORES)), trace=trace
    )
    # gather: y[c*2^21 + (16q+s)*2^14 + f] = O_q[c*16+s, f]
    outs = [res.results[q]["y"].reshape(NCORES, 16, F) for q in range(NCORES)]
    full = np.stack(outs, axis=1)  # (c, q, s, f)
    return full.reshape(NCORES * LOCAL), res


def kernel(Hamiltonian: np.ndarray) -> np.ndarray:
    y, _ = run(Hamiltonian, trace=False)
    return y

